# revision 1
# baseline (speedup 1.0000x reference)
"""Expert-parallel BruteForce MoE kernel for 8 TRN2 NeuronCores.

Model: N=1024 tokens, D=512 d_model, H=2048 d_hidden, E=8 experts, top-K=2.
  logits = inp @ gate_w.T + gate_b ; top2 -> softmax scores
  y(tok,e) = gelu(x @ w1[e].T + b1[e]) @ w2[e].T + b2[e]
  out = LN( sum_k score_k * y(tok, e_k) )

Strategy (exact, static shapes): core e owns expert e. Every core computes
the gate for all tokens and derives G[:, e] = per-token weight of expert e
(softmax score if e is in that token's top-2, else 0). Core e then computes
Z_e = G[:, e] * (gelu(X @ w1[e].T + b1[e]) @ w2[e].T + b2[e]) for ALL 1024
tokens, dense.  sum_e Z_e equals the routed-and-combined MoE output.

Pipelining: tokens are processed in two halves. For each half: layer-1 (all
16 h-chunks) -> layer-2 -> gate-scale -> ReduceScatter -> LayerNorm + store
of that half's 64-token shard. The first half's RS+LN overlap the second
half's compute. Host reassembles the shards.

DMA: the gate weights, b1 (pre-transposed to [128,16]) and the first token
half are packed into ONE DRAM tensor ("xg") so the critical head is a
single issue+transfer; w1T is split into two k-halves with alternating
accumulation start order so layer-1 begins as soon as the first half lands.

Matmul dtype float32r: byte-compatible with f32, 4x PE rate vs plain f32
(1 cycle/row for moving dim >= 256) at ~tf32 accuracy (2e-4 rel err e2e).
The gate consumes the same bytes via f32 bitcast views so top-2 selection
matches the reference exactly. gelu is computed as 0.5*t*(1+erf(t/sqrt2))
so the whole kernel uses one ACT table set (erf+sigmoid); LN rsqrt is
Newton on DVE (no sqrt table).
"""

import numpy as np

import concourse.bass as bass
import concourse.bacc as bacc
import concourse.tile as tile
from concourse import mybir
from concourse import bass_utils

E, D, H, K, N = 8, 512, 2048, 2, 1024
P = 128
EPS = 1e-5
NEG_BIG = -1e30
RSQRT2 = 0.7071067811865476

KC = D // P      # 4  contraction chunks over d_model
HC = H // P      # 16 chunks over d_hidden
TC = N // P      # 8  token chunks of 128
TW = 512         # tokens per pipeline half (= moving width for layer-1)
NTW = N // TW    # 2
TCH = TW // P    # 4  token chunks per half
SH = TW // E     # 64: tokens per core per RS half

F32 = mybir.dt.float32
F32R = mybir.dt.float32r

XOFF = E + HC            # 24: xg cols = [gwT(8) | b1p(16) | x half0 (512)]
XGW = XOFF + TW          # 536
# aux layout: [b2(512), lnw(512), lnb(512), gb(8), sel(8)]
AUXN = 3 * D + 2 * E


def _chunked(dram, kc, p=P):
    """AP view of a [kc*P, M] DRAM tensor as [P, kc, M] (partition-major)."""
    m = dram.shape[1]
    return bass.AP(tensor=dram[:, :].tensor, offset=0,
                   ap=[[m, p], [p * m, kc], [1, m]])


def _bcast(ap, p=P):
    """AP that reads `ap` (a 1-D DRAM view) replicated across p partitions."""
    return bass.AP(tensor=ap.tensor, offset=ap.offset, ap=[[0, p]] + list(ap.ap))


def build_nc(mm_dtype=F32R, single_core=False):
    """Build the SPMD program (same on all 8 cores; per-core data differs).

    single_core=True replaces the collectives with local DMAs so TimelineSim
    (single-core, no collectives) can time the kernel; numerics differ.
    """
    nc = bacc.Bacc("TRN2", target_bir_lowering=False, debug=False,
                   num_devices=1 if single_core else E)
    MM = mm_dtype

    # ---- per-core external inputs ----
    xg = nc.dram_tensor("xg", [D, XGW], MM, kind="ExternalInput")   # packed head
    xTb = nc.dram_tensor("xTb", [D, TW], MM, kind="ExternalInput")  # x half1
    w1T = nc.dram_tensor("w1T", [D, H], MM, kind="ExternalInput")   # w1[e].T
    w2T = nc.dram_tensor("w2T", [H, D], MM, kind="ExternalInput")   # w2[e].T
    aux = nc.dram_tensor("aux", [AUXN], F32, kind="ExternalInput")  # packed vectors
    # rows 0:64 = tokens [c*64, (c+1)*64), rows 64:128 = [512+c*64, 512+(c+1)*64)
    out = nc.dram_tensor("out", [P, D], F32, kind="ExternalOutput")

    # internal DRAM for the chunked collective (separate tensors so the
    # first RS only depends on the first half's writes)
    zdr = [nc.dram_tensor(f"zdram{i}", [TW, D], F32) for i in range(NTW)]
    zrd = [nc.dram_tensor(f"zred{i}", [SH, D], F32) for i in range(NTW)]

    with tile.TileContext(nc) as tc:
        with (
            tc.tile_pool(name="persist", bufs=1) as persist,
            tc.tile_pool(name="work", bufs=4) as work,
            tc.tile_pool(name="zout", bufs=3) as zout,
            tc.tile_pool(name="psg", bufs=1, space="PSUM") as psg,
            tc.tile_pool(name="ps1", bufs=5, space="PSUM") as ps1,
            tc.tile_pool(name="ps2", bufs=2, space="PSUM") as ps2,
        ):
            xf = (lambda ap: ap.bitcast(F32)) if MM == F32R else (lambda ap: ap)

            # ---- persistent SBUF loads, ordered by first use ----
            xg_sb = persist.tile([P, KC, XGW], MM, tag="xg")
            xg_view = _chunked(xg, KC)
            w1T_sb = persist.tile([P, KC, H], MM, tag="w1T")
            w1T_view = _chunked(w1T, KC)
            for k in range(KC):
                nc.sync.dma_start(out=xg_sb[:, k:k + 1, :], in_=xg_view[:, k:k + 1, :])
                nc.sync.dma_start(out=w1T_sb[:, k:k + 1, :], in_=w1T_view[:, k:k + 1, :])

            xTb_sb = persist.tile([P, KC, TW], MM, tag="xTb")
            nc.sync.dma_start(out=xTb_sb, in_=_chunked(xTb, KC))

            w2T_sb = persist.tile([P, HC, D], MM, tag="w2T")
            w2T_view = _chunked(w2T, HC)
            HH = HC // 2
            nc.sync.dma_start(out=w2T_sb[:, 0:HH, :], in_=w2T_view[:, 0:HH, :])
            aux_sb = persist.tile([P, AUXN], F32, tag="aux")
            nc.sync.dma_start(out=aux_sb, in_=_bcast(aux[:]))
            nc.sync.dma_start(out=w2T_sb[:, HH:HC, :], in_=w2T_view[:, HH:HC, :])
            b2_sb = aux_sb[:, 0:D]
            lnw_sb = aux_sb[:, D:2 * D]
            lnb_sb = aux_sb[:, 2 * D:3 * D]
            gb_sb = aux_sb[:, 3 * D:3 * D + E]
            sel_sb = aux_sb[:, 3 * D + E:3 * D + 2 * E]

            eps_sb = persist.tile([P, 1], F32, tag="eps")
            nc.vector.memset(eps_sb, EPS)
            # first ACT op: pulls the single erf/sigmoid table in early
            warm = persist.tile([P, 1], F32, tag="warm")
            nc.scalar.activation(warm, eps_sb, mybir.ActivationFunctionType.Erf)

            # b1 views from the packed xg (chunk 0, cols 8:24) + b1/sqrt2
            b1_sb = xf(xg_sb[:, 0, E:E + HC])                  # [P, 16]
            b1h_sb = persist.tile([P, HC], F32, tag="b1h")
            nc.vector.tensor_scalar(
                out=b1h_sb, in0=b1_sb, scalar1=RSQRT2, scalar2=None,
                op0=mybir.AluOpType.mult,
            )

            def xcol(t):
                """lhsT view of token chunk t for the gate, per k."""
                if t < TCH:
                    return lambda k: xf(
                        xg_sb[:, k, XOFF + t * P:XOFF + (t + 1) * P])
                return lambda k: xf(
                    xTb_sb[:, k, (t - TCH) * P:(t - TCH + 1) * P])

            # ---- gate matmuls: logits for all tokens (full f32) ----
            La = persist.tile([P, TC, E], F32, tag="La")
            for t in range(TC):
                pg = psg.tile([P, E], F32, tag="psg")
                col = xcol(t)
                for k in range(KC):
                    nc.tensor.matmul(
                        pg,
                        lhsT=col(k),
                        rhs=xf(xg_sb[:, k, 0:E]),
                        start=(k == 0),
                        stop=(k == KC - 1),
                    )
                nc.vector.tensor_copy(out=La[:, t, :], in_=pg)

            def layer1(tw, g1):
                rhs_of = (lambda k: xg_sb[:, k, XOFF:XOFF + TW]) if tw == 0 \
                    else (lambda k: xTb_sb[:, k, :])
                for h in range(HC):
                    p1 = ps1.tile([P, TW], F32, tag="ps1")
                    for j, k in enumerate(range(KC)):
                        nc.tensor.matmul(
                            p1,
                            lhsT=w1T_sb[:, k, h * P:(h + 1) * P],
                            rhs=rhs_of(k),
                            start=(j == 0),
                            stop=(j == KC - 1),
                        )
                    # gelu(t) = 0.5*(t)*(1+erf(t/sqrt2)), t = p1 + b1
                    er = work.tile([P, TW], F32, tag="er")
                    nc.scalar.activation(
                        er, p1, mybir.ActivationFunctionType.Erf,
                        bias=b1h_sb[:, h:h + 1], scale=RSQRT2,
                    )
                    ht = work.tile([P, TW], F32, tag="ht")
                    nc.vector.tensor_scalar(
                        out=ht, in0=p1, scalar1=b1_sb[:, h:h + 1], scalar2=0.5,
                        op0=mybir.AluOpType.add, op1=mybir.AluOpType.mult,
                    )
                    nc.vector.scalar_tensor_tensor(
                        out=g1[:, h, :], in0=er, scalar=1.0, in1=ht,
                        op0=mybir.AluOpType.add, op1=mybir.AluOpType.mult,
                    )

            def gate_chain():
                # top-2 mask math on [P, TC, E]; emitted after the first
                # layer-1 half so the ACT sigmoid never blocks gelu evictions
                X = mybir.AxisListType.X
                nc.vector.tensor_tensor(
                    out=La, in0=La,
                    in1=gb_sb[:, None, :].to_broadcast((P, TC, E)),
                    op=mybir.AluOpType.add,
                )
                v1 = work.tile([P, TC], F32, tag="v1")
                nc.vector.reduce_max(out=v1, in_=La, axis=X)
                eq1 = work.tile([P, TC, E], F32, tag="eq1")
                nc.vector.tensor_tensor(
                    out=eq1, in0=La, in1=v1[:, :, None].to_broadcast((P, TC, E)),
                    op=mybir.AluOpType.is_equal,
                )
                Lm = work.tile([P, TC, E], F32, tag="Lm")
                nc.vector.scalar_tensor_tensor(
                    out=Lm, in0=eq1, scalar=NEG_BIG, in1=La,
                    op0=mybir.AluOpType.mult, op1=mybir.AluOpType.add,
                )
                v2 = work.tile([P, TC], F32, tag="v2")
                nc.vector.reduce_max(out=v2, in_=Lm, axis=X)
                eq2 = work.tile([P, TC, E], F32, tag="eq2")
                nc.vector.tensor_tensor(
                    out=eq2, in0=Lm, in1=v2[:, :, None].to_broadcast((P, TC, E)),
                    op=mybir.AluOpType.is_equal,
                )
                s2 = work.tile([P, TC], F32, tag="s2")
                nc.vector.tensor_sub(s2, v2, v1)
                nc.scalar.activation(s2, s2, mybir.ActivationFunctionType.Sigmoid)
                s1 = work.tile([P, TC], F32, tag="s1")
                nc.vector.tensor_scalar(
                    out=s1, in0=s2, scalar1=-1.0, scalar2=1.0,
                    op0=mybir.AluOpType.mult, op1=mybir.AluOpType.add,
                )
                A1 = work.tile([P, TC, E], F32, tag="A1")
                nc.vector.tensor_mul(
                    A1, eq1, s1[:, :, None].to_broadcast((P, TC, E)))
                A2 = work.tile([P, TC, E], F32, tag="A2")
                nc.vector.tensor_mul(
                    A2, eq2, s2[:, :, None].to_broadcast((P, TC, E)))
                nc.vector.tensor_add(A1, A1, A2)
                nc.vector.tensor_mul(
                    A1, A1, sel_sb[:, None, :].to_broadcast((P, TC, E)))
                gcol = persist.tile([P, TC], F32, tag="gcol")
                nc.vector.reduce_sum(out=gcol, in_=A1, axis=X)
                return gcol

            def layer2(tw, g1, gcol):
                for tl in range(TCH):
                    t = tw * TCH + tl
                    p2 = ps2.tile([P, D], F32, tag="ps2")
                    for h in range(HC):
                        nc.tensor.matmul(
                            p2,
                            lhsT=g1[:, h, tl * P:(tl + 1) * P],
                            rhs=w2T_sb[:, h, :],
                            start=(h == 0),
                            stop=(h == HC - 1),
                        )
                    zt = zout.tile([P, D], F32, tag="zt")
                    nc.vector.tensor_add(zt, p2, b2_sb)
                    nc.vector.tensor_scalar(
                        out=zt, in0=zt, scalar1=gcol[:, t:t + 1], scalar2=None,
                        op0=mybir.AluOpType.mult,
                    )
                    nc.sync.dma_start(
                        out=zdr[tw][tl * P:(tl + 1) * P, :], in_=zt)
                if not single_core:
                    nc.gpsimd.collective_compute(
                        "ReduceScatter",
                        mybir.AluOpType.add,
                        replica_groups=[list(range(E))],
                        ins=[zdr[tw][:, :].opt()],
                        outs=[zrd[tw][:, :].opt()],
                    )

            zsb = persist.tile([P, D], F32, tag="zsb")

            def ln_half(half):
                """LayerNorm + store of this half's 64-token shard."""
                o = half * SH
                src = zdr[half][0:SH, :] if single_core else zrd[half][:, :]
                nc.sync.dma_start(out=zsb[o:o + SH, :], in_=src)
                z = zsb[o:o + SH, :]
                stats = work.tile([P, 6], F32, tag="stats")
                nc.vector.bn_stats(out=stats[0:SH], in_=z)
                mv = work.tile([P, 2], F32, tag="mv")
                nc.vector.bn_aggr(out=mv[0:SH], in_=stats[0:SH])
                # rstd via bit-hack + 3 Newton steps (no sqrt table needed)
                rstd = work.tile([P, 1], F32, tag="rstd")
                ve = work.tile([P, 1], F32, tag="ve")
                nc.vector.tensor_scalar(
                    out=ve[0:SH], in0=mv[0:SH, 1:2], scalar1=float(EPS),
                    scalar2=None, op0=mybir.AluOpType.add,
                )
                I32 = mybir.dt.int32
                nc.vector.tensor_scalar(
                    out=rstd[0:SH].bitcast(I32), in0=ve[0:SH].bitcast(I32),
                    scalar1=1, scalar2=None,
                    op0=mybir.AluOpType.arith_shift_right,
                )
                nc.vector.tensor_scalar(
                    out=rstd[0:SH].bitcast(I32), in0=rstd[0:SH].bitcast(I32),
                    scalar1=-1, scalar2=0x5F3759DF,
                    op0=mybir.AluOpType.mult, op1=mybir.AluOpType.add,
                )
                t1 = work.tile([P, 1], F32, tag="t1")
                for _ in range(3):        # y *= 1.5 - 0.5*v*y*y
                    nc.vector.tensor_mul(t1[0:SH], rstd[0:SH], rstd[0:SH])
                    nc.vector.tensor_mul(t1[0:SH], t1[0:SH], ve[0:SH])
                    nc.vector.tensor_scalar(
                        out=t1[0:SH], in0=t1[0:SH], scalar1=-0.5, scalar2=1.5,
                        op0=mybir.AluOpType.mult, op1=mybir.AluOpType.add,
                    )
                    nc.vector.tensor_mul(rstd[0:SH], rstd[0:SH], t1[0:SH])
                xn = work.tile([P, D], F32, tag="xn")
                nc.vector.tensor_scalar(
                    out=xn[0:SH], in0=z, scalar1=mv[0:SH, 0:1],
                    scalar2=rstd[0:SH],
                    op0=mybir.AluOpType.subtract, op1=mybir.AluOpType.mult,
                )
                nc.vector.tensor_mul(xn[0:SH], xn[0:SH], lnw_sb[0:SH])
                nc.vector.tensor_add(xn[0:SH], xn[0:SH], lnb_sb[0:SH])
                nc.sync.dma_start(out=out[o:o + SH, :], in_=xn[0:SH])

            # ---- pipelined halves ----
            g1a = persist.tile([P, HC, TW], MM, tag="g1a")
            g1b = persist.tile([P, HC, TW], MM, tag="g1b")
            layer1(0, g1a)
            gcol = gate_chain()
            layer2(0, g1a, gcol)
            layer1(1, g1b)
            ln_half(0)
            layer2(1, g1b, gcol)
            ln_half(1)

    nc.compile()
    return nc


_CACHE = {}


def _get_nc(key, mm_dtype):
    if key not in _CACHE:
        _CACHE[key] = build_nc(mm_dtype)
    return _CACHE[key]


MM_DTYPE = "f32r"  # "f32" | "f32r"


def make_in_maps(inputs, mm_np=np.float32):
    inp = np.asarray(inputs["inp"], dtype=np.float32)
    gate_w = np.asarray(inputs["gate_w"], dtype=np.float32)
    gate_b = np.asarray(inputs["gate_b"], dtype=np.float32)
    w1 = np.asarray(inputs["w1"], dtype=np.float32)
    b1 = np.asarray(inputs["b1"], dtype=np.float32)
    w2 = np.asarray(inputs["w2"], dtype=np.float32)
    b2 = np.asarray(inputs["b2"], dtype=np.float32)
    ln_w = np.asarray(inputs["ln_w"], dtype=np.float32)
    ln_b = np.asarray(inputs["ln_b"], dtype=np.float32)

    xT = np.ascontiguousarray(inp.T)                      # [D, N]
    gwT = np.ascontiguousarray(gate_w.T)                  # [D, E]
    eye = np.eye(E, dtype=np.float32)

    in_maps = []
    for c in range(E):
        xgv = np.zeros((D, XGW), np.float32)
        xgv[:, 0:E] = gwT
        # b1 pre-transposed into chunk 0: b1p[p, h] = b1[c][h*128+p]
        xgv[0:P, E:XOFF] = b1[c].reshape(HC, P).T
        xgv[:, XOFF:XGW] = xT[:, 0:TW]
        auxv = np.concatenate([b2[c], ln_w, ln_b, gate_b, eye[c]]).astype(np.float32)
        in_maps.append({
            "xg": xgv.astype(mm_np),
            "xTb": np.ascontiguousarray(xT[:, TW:N]).astype(mm_np),
            "w1T": np.ascontiguousarray(w1[c].T).astype(mm_np),   # [D, H]
            "w2T": np.ascontiguousarray(w2[c].T).astype(mm_np),   # [H, D]
            "aux": auxv,
        })
    return in_maps


def kernel(**inputs):
    mm_dt = F32R if MM_DTYPE == "f32r" else F32
    nc = _get_nc(MM_DTYPE, mm_dt)
    in_maps = make_in_maps(inputs)
    res = bass_utils.run_bass_kernel_spmd(nc, in_maps, core_ids=list(range(E)))
    # core c's output rows 0:64 are tokens [c*64,(c+1)*64); rows 64:128 are
    # tokens [512+c*64, 512+(c+1)*64)
    full = np.empty((N, D), np.float32)
    for c in range(E):
        o = res.results[c]["out"]
        full[c * SH:(c + 1) * SH] = o[0:SH]
        full[TW + c * SH:TW + (c + 1) * SH] = o[SH:P]
    return full



# revision 16
# speedup vs baseline: 1.2333x; 1.2333x over previous
"""Expert-parallel BruteForce MoE kernel for 8 TRN2 NeuronCores.

Model: N=1024 tokens, D=512 d_model, H=2048 d_hidden, E=8 experts, top-K=2.
  logits = inp @ gate_w.T + gate_b ; top2 -> softmax scores
  y(tok,e) = gelu(x @ w1[e].T + b1[e]) @ w2[e].T + b2[e]
  out = LN( sum_k score_k * y(tok, e_k) )

Strategy (exact, static shapes): core e owns expert e. Every core computes
the gate for all tokens and derives G[:, e] = per-token weight of expert e
(softmax score if e is in that token's top-2, else 0). Core e then computes
Z_e = G[:, e] * (gelu(X @ w1[e].T + b1[e]) @ w2[e].T + b2[e]) for ALL 1024
tokens, dense.  sum_e Z_e equals the routed-and-combined MoE output.

Dtypes: the expert MLP (x, w1, gelu output, w2) runs in bf16 (1 cycle/row on
the PE, half the HBM traffic of f32; ~3e-3 end-to-end rel err, well under
the 2e-2 gate).  The gate keeps a separate f32r copy of x so top-2 selection
matches the reference bit-for-bit at selection boundaries (min top2/top3
logit gap is ~2e-4; bf16 logits would flip some selections).

Engine placement: PE does only matmuls (gate + both layers, ~57us busy).
The whole gelu epilogue is ONE fused ACT instruction per tile
(g1 = Gelu(psum + b1[h]), PSUM->SBUF, bf16 out).  The gate softmax uses
tanh (sigmoid(x) = 0.5 + 0.5 tanh(x/2)) so gelu+tanh come from a single ACT
table set.  DVE only does the tiny gate mask math, layer-2 epilogues and the
LayerNorm (Newton rsqrt, no sqrt table).

DMA is issued in consumption order with fine chunks so the PE never waits:
x-bf16 half0 (4 quarter-chunks) -> w1 (16 h-chunks) -> xg f32r (gate) ->
w2 (16 h-chunks) -> x-bf16 half1.  Layer-1 starts ~1.5us in.

Pipelining: layer1(half0) -> gate -> layer2(half0) -> layer1(half1) ->
layer2(half1), with each half's gate-scale + ReduceScatter + LayerNorm
overlapping the next half's compute.  Host reassembles the 8 shards.
"""

import numpy as np
import ml_dtypes

import concourse.bass as bass
import concourse.bacc as bacc
import concourse.tile as tile
from concourse import mybir
from concourse import bass_utils

E, D, H, K, N = 8, 512, 2048, 2, 1024
P = 128
EPS = 1e-5

KC = D // P      # 4  contraction chunks over d_model
HC = H // P      # 16 chunks over d_hidden
TC = N // P      # 8  token chunks of 128
TW = 512         # tokens per pipeline half (= moving width for layer-1)
NTW = N // TW    # 2
TCH = TW // P    # 4  token chunks per half
SH = TW // E     # 64: tokens per core per RS half
NEG_BIG = -1e30

F32 = mybir.dt.float32
F32R = mybir.dt.float32r
BF16 = mybir.dt.bfloat16

XOFF = E                 # xg cols = [gwT(8) | xT f32 (1024)]
XGW = XOFF + N           # 1032
# aux layout: [gb(8), sel(8), b2(512), lnw(512), lnb(512)]
AUXN = 3 * D + 2 * E


def _chunked(dram, kc, p=P):
    """AP view of a [kc*P, M] DRAM tensor as [P, kc, M] (partition-major)."""
    m = dram.shape[1]
    return bass.AP(tensor=dram[:, :].tensor, offset=0,
                   ap=[[m, p], [p * m, kc], [1, m]])


def _bcast(ap, p=P):
    """AP that reads `ap` (a 1-D DRAM view) replicated across p partitions."""
    return bass.AP(tensor=ap.tensor, offset=ap.offset, ap=[[0, p]] + list(ap.ap))


def build_nc(mm_dtype=F32R, single_core=False):
    """Build the SPMD program (same on all 8 cores; per-core data differs).

    single_core=True replaces the collectives with local DMAs so TimelineSim
    (single-core, no collectives) can time the kernel; numerics differ.
    mm_dtype sets the GATE matmul dtype (f32r default); the MLP is bf16.
    """
    nc = bacc.Bacc("TRN2", target_bir_lowering=False, debug=False,
                   num_devices=1 if single_core else E)
    MM = mm_dtype

    # ---- per-core external inputs ----
    # w1pk is host-packed so each 2-h-chunk DMA reads 2KB contiguous per
    # partition (512B strided pieces pay a 2x DMA penalty):
    #   w1pk[p, g, k, j] = w1[e].T[k*128+p, g*256+j]
    xg = nc.dram_tensor("xg", [D, XGW], MM, kind="ExternalInput")   # gate pack
    xbf = nc.dram_tensor("xbf", [D, N], BF16, kind="ExternalInput")  # xT bf16
    w1pk = nc.dram_tensor("w1pk", [P, HC // 2, KC, 2 * P], BF16,
                          kind="ExternalInput")
    w2b = nc.dram_tensor("w2b", [H, D], BF16, kind="ExternalInput")  # w2[e].T
    b1p = nc.dram_tensor("b1p", [P, HC], F32, kind="ExternalInput")  # b1 packed
    aux = nc.dram_tensor("aux", [AUXN], F32, kind="ExternalInput")  # packed vecs
    # rows 0:64 = tokens [c*64, (c+1)*64), rows 64:128 = [512+c*64, 512+(c+1)*64)
    out = nc.dram_tensor("out", [P, D], F32, kind="ExternalOutput")

    # internal DRAM for the chunked collective (separate tensors so the
    # first RS only depends on the first half's writes)
    zdr = [nc.dram_tensor(f"zdram{i}", [TW, D], F32) for i in range(NTW)]
    zrd = [nc.dram_tensor(f"zred{i}", [SH, D], F32) for i in range(NTW)]

    with tile.TileContext(nc) as tc:
        with (
            tc.tile_pool(name="persist", bufs=1) as persist,
            tc.tile_pool(name="work", bufs=4) as work,
            tc.tile_pool(name="zout", bufs=3) as zout,
            tc.tile_pool(name="psg", bufs=2, space="PSUM") as psg,
            tc.tile_pool(name="ps1", bufs=4, space="PSUM") as ps1,
            tc.tile_pool(name="ps2", bufs=2, space="PSUM") as ps2,
        ):
            xf = (lambda ap: ap.bitcast(F32)) if MM == F32R else (lambda ap: ap)

            # ---- persistent SBUF tiles ----
            xbf_sb = persist.tile([P, KC, N], BF16, tag="xbf")
            xbf_view = _chunked(xbf, KC)
            w1_sb = persist.tile([P, HC // 2, KC, 2 * P], BF16, tag="w1pk")
            w2_sb = persist.tile([P, HC, D], BF16, tag="w2b")
            w2_view = _chunked(w2b, HC)
            xg_sb = persist.tile([P, KC, XGW], MM, tag="xg")
            xg_view = _chunked(xg, KC)
            aux_sb = persist.tile([P, AUXN], F32, tag="aux")
            b1_sb = persist.tile([P, HC], F32, tag="b1p")

            gb_sb = aux_sb[:, 0:E]
            sel_sb = aux_sb[:, E:2 * E]
            b2_sb = aux_sb[:, 2 * E:2 * E + D]
            lnw_sb = aux_sb[:, 2 * E + D:2 * E + 2 * D]
            lnb_sb = aux_sb[:, 2 * E + 2 * D:2 * E + 3 * D]

            # ---- DMA issue order == consumption order.  ~650ns of HWDGE
            # descriptor-gen per dma_start is serial across DMAs, so chunks
            # are sized to stay just ahead of PE consumption, no finer; the
            # big aux broadcasts are placed outside the weight supply path. --
            nc.sync.dma_start(out=xbf_sb[:, :, 0:TW],       # x bf16 half 0
                              in_=xbf_view[:, :, 0:TW])
            nc.sync.dma_start(out=w1_sb[:, 0:1], in_=w1pk[:, 0:1])
            nc.sync.dma_start(out=b1_sb, in_=b1p[:, :])
            nc.sync.dma_start(out=aux_sb[:, 0:2 * E],       # gb + sel
                              in_=_bcast(aux[0:2 * E]))
            for g in range(1, HC // 2):  # rest of w1 (layer-1 lhsT)
                nc.sync.dma_start(out=w1_sb[:, g:g + 1], in_=w1pk[:, g:g + 1])
            for k in range(0, KC, 2):  # gate pack (f32r): gwT + full xT
                nc.sync.dma_start(out=xg_sb[:, k:k + 2, :],
                                  in_=xg_view[:, k:k + 2, :])
            for h in range(0, HC, 2):  # w2 in 8 chunks (layer-2 rhs)
                if h == 4:             # b2 lands before the first epilogue
                    nc.sync.dma_start(out=aux_sb[:, 2 * E:2 * E + D],
                                      in_=_bcast(aux[2 * E:2 * E + D]))
                nc.sync.dma_start(out=w2_sb[:, h:h + 2, :],
                                  in_=w2_view[:, h:h + 2, :])
            nc.sync.dma_start(out=xbf_sb[:, :, TW:N],       # x bf16 half 1
                              in_=xbf_view[:, :, TW:N])
            nc.sync.dma_start(out=aux_sb[:, 2 * E + D:],    # lnw + lnb
                              in_=_bcast(aux[2 * E + D:]))

            eps_sb = persist.tile([P, 1], F32, tag="eps")
            nc.vector.memset(eps_sb, EPS)
            # first ACT op: pulls the single gelu/tanh table in early
            warm = persist.tile([P, 1], F32, tag="warm")
            nc.scalar.activation(warm, eps_sb, mybir.ActivationFunctionType.Gelu)

            # PE pstate warm-up: the tensor engine ramps 0.65 -> 1.2 -> 2.4GHz
            # over ~3us of continuous execution.  Burn the DMA head on dummy
            # matmuls over a memset tile so real matmuls start at full clock.
            wu = persist.tile([P, P], F32, tag="wu")
            nc.vector.memset(wu, 0.0)
            for _ in range(12):
                pw = psg.tile([P, 64], F32, tag="psg")
                nc.tensor.matmul(pw, lhsT=wu, rhs=wu[:, 0:64],
                                 start=True, stop=True)

            # ---- layer 1: p1 = x @ w1.T ; g1 = Gelu(p1 + b1)  (one ACT op) --
            def layer1(tw, g1):
                for h in range(HC):
                    p1 = ps1.tile([P, TW], F32, tag="ps1")
                    for j, k in enumerate(range(KC)):
                        nc.tensor.matmul(
                            p1,
                            lhsT=w1_sb[:, h // 2, k,
                                       (h % 2) * P:(h % 2 + 1) * P],
                            rhs=xbf_sb[:, k, tw * TW:(tw + 1) * TW],
                            start=(j == 0),
                            stop=(j == KC - 1),
                        )
                    nc.scalar.activation(
                        g1[:, h, :], p1, mybir.ActivationFunctionType.Gelu,
                        bias=b1_sb[:, h:h + 1], scale=1.0,
                    )

            # ---- gate: logits for all tokens (f32r = tf32-ish accuracy) ----
            La = persist.tile([P, TC, E], F32, tag="La")

            def gate_mm():
                for t in range(TC):
                    pg = psg.tile([P, E], F32, tag="psg")
                    for k in range(KC):
                        nc.tensor.matmul(
                            pg,
                            lhsT=xf(xg_sb[:, k, XOFF + t * P:XOFF + (t + 1) * P]),
                            rhs=xf(xg_sb[:, k, 0:E]),
                            start=(k == 0),
                            stop=(k == KC - 1),
                        )
                    nc.vector.tensor_copy(out=La[:, t, :], in_=pg)

            def gate_chain():
                # top-2 mask math on [P, TC, E] -> gcol[:, t] = this core's
                # expert weight for each token (0 if not in top-2)
                X = mybir.AxisListType.X
                nc.vector.tensor_tensor(
                    out=La, in0=La,
                    in1=gb_sb[:, None, :].to_broadcast((P, TC, E)),
                    op=mybir.AluOpType.add,
                )
                v1 = work.tile([P, TC], F32, tag="v1")
                nc.vector.reduce_max(out=v1, in_=La, axis=X)
                eq1 = work.tile([P, TC, E], F32, tag="eq1")
                nc.vector.tensor_tensor(
                    out=eq1, in0=La, in1=v1[:, :, None].to_broadcast((P, TC, E)),
                    op=mybir.AluOpType.is_equal,
                )
                Lm = work.tile([P, TC, E], F32, tag="Lm")
                nc.vector.scalar_tensor_tensor(
                    out=Lm, in0=eq1, scalar=NEG_BIG, in1=La,
                    op0=mybir.AluOpType.mult, op1=mybir.AluOpType.add,
                )
                v2 = work.tile([P, TC], F32, tag="v2")
                nc.vector.reduce_max(out=v2, in_=Lm, axis=X)
                eq2 = work.tile([P, TC, E], F32, tag="eq2")
                nc.vector.tensor_tensor(
                    out=eq2, in0=Lm, in1=v2[:, :, None].to_broadcast((P, TC, E)),
                    op=mybir.AluOpType.is_equal,
                )
                # s2 = sigmoid(v2-v1) = 0.5 + 0.5*tanh((v2-v1)/2): tanh shares
                # the gelu ACT table so only one table set is ever loaded
                th = work.tile([P, TC], F32, tag="th")
                nc.vector.tensor_sub(th, v2, v1)
                nc.scalar.activation(
                    th, th, mybir.ActivationFunctionType.Tanh, scale=0.5)
                s2 = work.tile([P, TC], F32, tag="s2")
                nc.vector.tensor_scalar(
                    out=s2, in0=th, scalar1=0.5, scalar2=0.5,
                    op0=mybir.AluOpType.mult, op1=mybir.AluOpType.add,
                )
                s1 = work.tile([P, TC], F32, tag="s1")
                nc.vector.tensor_scalar(
                    out=s1, in0=th, scalar1=-0.5, scalar2=0.5,
                    op0=mybir.AluOpType.mult, op1=mybir.AluOpType.add,
                )
                A1 = work.tile([P, TC, E], F32, tag="A1")
                nc.vector.tensor_mul(
                    A1, eq1, s1[:, :, None].to_broadcast((P, TC, E)))
                A2 = work.tile([P, TC, E], F32, tag="A2")
                nc.vector.tensor_mul(
                    A2, eq2, s2[:, :, None].to_broadcast((P, TC, E)))
                nc.vector.tensor_add(A1, A1, A2)
                nc.vector.tensor_mul(
                    A1, A1, sel_sb[:, None, :].to_broadcast((P, TC, E)))
                gcol = persist.tile([P, TC], F32, tag="gcol")
                nc.vector.reduce_sum(out=gcol, in_=A1, axis=X)
                return gcol

            def layer2(tw, g1, gcol):
                for tl in range(TCH):
                    t = tw * TCH + tl
                    p2 = ps2.tile([P, D], F32, tag="ps2")
                    for h in range(HC):
                        nc.tensor.matmul(
                            p2,
                            lhsT=g1[:, h, tl * P:(tl + 1) * P],
                            rhs=w2_sb[:, h, :],
                            start=(h == 0),
                            stop=(h == HC - 1),
                        )
                    zt = zout.tile([P, D], F32, tag="zt")
                    nc.vector.tensor_add(zt, p2, b2_sb)
                    nc.vector.tensor_scalar(
                        out=zt, in0=zt, scalar1=gcol[:, t:t + 1], scalar2=None,
                        op0=mybir.AluOpType.mult,
                    )
                    nc.sync.dma_start(
                        out=zdr[tw][tl * P:(tl + 1) * P, :], in_=zt)
                if not single_core:
                    nc.gpsimd.collective_compute(
                        "ReduceScatter",
                        mybir.AluOpType.add,
                        replica_groups=[list(range(E))],
                        ins=[zdr[tw][:, :].opt()],
                        outs=[zrd[tw][:, :].opt()],
                    )

            zsb = persist.tile([P, D], F32, tag="zsb")

            def ln_half(half):
                """LayerNorm + store of this half's 64-token shard."""
                o = half * SH
                src = zdr[half][0:SH, :] if single_core else zrd[half][:, :]
                nc.sync.dma_start(out=zsb[o:o + SH, :], in_=src)
                z = zsb[o:o + SH, :]
                stats = work.tile([P, 6], F32, tag="stats")
                nc.vector.bn_stats(out=stats[0:SH], in_=z)
                mv = work.tile([P, 2], F32, tag="mv")
                nc.vector.bn_aggr(out=mv[0:SH], in_=stats[0:SH])
                # rstd via bit-hack + 3 Newton steps (no sqrt table needed)
                rstd = work.tile([P, 1], F32, tag="rstd")
                ve = work.tile([P, 1], F32, tag="ve")
                nc.vector.tensor_scalar(
                    out=ve[0:SH], in0=mv[0:SH, 1:2], scalar1=float(EPS),
                    scalar2=None, op0=mybir.AluOpType.add,
                )
                I32 = mybir.dt.int32
                nc.vector.tensor_scalar(
                    out=rstd[0:SH].bitcast(I32), in0=ve[0:SH].bitcast(I32),
                    scalar1=1, scalar2=None,
                    op0=mybir.AluOpType.arith_shift_right,
                )
                nc.vector.tensor_scalar(
                    out=rstd[0:SH].bitcast(I32), in0=rstd[0:SH].bitcast(I32),
                    scalar1=-1, scalar2=0x5F3759DF,
                    op0=mybir.AluOpType.mult, op1=mybir.AluOpType.add,
                )
                t1 = work.tile([P, 1], F32, tag="t1")
                for _ in range(3):        # y *= 1.5 - 0.5*v*y*y
                    nc.vector.tensor_mul(t1[0:SH], rstd[0:SH], rstd[0:SH])
                    nc.vector.tensor_mul(t1[0:SH], t1[0:SH], ve[0:SH])
                    nc.vector.tensor_scalar(
                        out=t1[0:SH], in0=t1[0:SH], scalar1=-0.5, scalar2=1.5,
                        op0=mybir.AluOpType.mult, op1=mybir.AluOpType.add,
                    )
                    nc.vector.tensor_mul(rstd[0:SH], rstd[0:SH], t1[0:SH])
                xn = work.tile([P, D], F32, tag="xn")
                nc.vector.tensor_scalar(
                    out=xn[0:SH], in0=z, scalar1=mv[0:SH, 0:1],
                    scalar2=rstd[0:SH],
                    op0=mybir.AluOpType.subtract, op1=mybir.AluOpType.mult,
                )
                nc.vector.tensor_mul(xn[0:SH], xn[0:SH], lnw_sb[0:SH])
                nc.vector.tensor_add(xn[0:SH], xn[0:SH], lnb_sb[0:SH])
                nc.sync.dma_start(out=out[o:o + SH, :], in_=xn[0:SH])

            # ---- pipelined halves ----
            g1a = persist.tile([P, HC, TW], BF16, tag="g1a")
            g1b = persist.tile([P, HC, TW], BF16, tag="g1b")
            layer1(0, g1a)
            gate_mm()                   # xg f32r lands during layer1(0)
            gcol = gate_chain()
            layer2(0, g1a, gcol)
            layer1(1, g1b)
            ln_half(0)
            layer2(1, g1b, gcol)
            ln_half(1)

    nc.compile()
    return nc


_CACHE = {}


def _get_nc(key, mm_dtype):
    if key not in _CACHE:
        _CACHE[key] = build_nc(mm_dtype)
    return _CACHE[key]


MM_DTYPE = "f32r"  # gate matmul dtype: "f32" | "f32r"


def make_in_maps(inputs, mm_np=np.float32):
    inp = np.asarray(inputs["inp"], dtype=np.float32)
    gate_w = np.asarray(inputs["gate_w"], dtype=np.float32)
    gate_b = np.asarray(inputs["gate_b"], dtype=np.float32)
    w1 = np.asarray(inputs["w1"], dtype=np.float32)
    b1 = np.asarray(inputs["b1"], dtype=np.float32)
    w2 = np.asarray(inputs["w2"], dtype=np.float32)
    b2 = np.asarray(inputs["b2"], dtype=np.float32)
    ln_w = np.asarray(inputs["ln_w"], dtype=np.float32)
    ln_b = np.asarray(inputs["ln_b"], dtype=np.float32)

    xT = np.ascontiguousarray(inp.T)                      # [D, N]
    gwT = np.ascontiguousarray(gate_w.T)                  # [D, E]
    eye = np.eye(E, dtype=np.float32)
    bf = ml_dtypes.bfloat16

    xgv = np.zeros((D, XGW), np.float32)
    xgv[:, 0:E] = gwT
    xgv[:, XOFF:XGW] = xT
    xbfv = xT.astype(bf)

    in_maps = []
    for c in range(E):
        auxv = np.concatenate(
            [gate_b, eye[c], b2[c], ln_w, ln_b]).astype(np.float32)
        # w1pk[p, g, k, j] = w1[c].T[k*128+p, g*256+j]
        w1pk = np.ascontiguousarray(
            w1[c].T.reshape(KC, P, HC // 2, 2 * P).transpose(1, 2, 0, 3))
        in_maps.append({
            "xg": xgv.astype(mm_np),
            "xbf": xbfv,
            "w1pk": w1pk.astype(bf),
            "w2b": np.ascontiguousarray(w2[c].T).astype(bf),   # [H, D]
            "b1p": np.ascontiguousarray(b1[c].reshape(HC, P).T),
            "aux": auxv,
        })
    return in_maps


def kernel(**inputs):
    mm_dt = F32R if MM_DTYPE == "f32r" else F32
    nc = _get_nc(MM_DTYPE, mm_dt)
    in_maps = make_in_maps(inputs)
    res = bass_utils.run_bass_kernel_spmd(nc, in_maps, core_ids=list(range(E)))
    # core c's output rows 0:64 are tokens [c*64,(c+1)*64); rows 64:128 are
    # tokens [512+c*64, 512+(c+1)*64)
    full = np.empty((N, D), np.float32)
    for c in range(E):
        o = res.results[c]["out"]
        full[c * SH:(c + 1) * SH] = o[0:SH]
        full[TW + c * SH:TW + (c + 1) * SH] = o[SH:P]
    return full
